# revision 15
# baseline (speedup 1.0000x reference)
"""Trainium2 Bass kernel for nn_DualAxisAggAttn (dual-axis aggregation attention).

Reference semantics per batch image x[C=256, H=64, W=64], twice (W axis then H axis):
  qkv = conv1x1(x) -> {q:[1], k:[C], v:[C]};  s = softmax_axis(q)
  ctx[c,a] = sum_r k*s;  out = x + sigmoid(v) * ctx_bcast;  y = conv1x1(out)

Distribution: data-parallel over batch (16 images -> 2 per NeuronCore x 8 cores).

Structure (v4):
  - STAGE COLLAPSE: every stage-H op on x_w = WfW @ out_W is linear in
    channels, so WfW is folded into stage-H's weights on the HOST
    (qH' = qH@WfW, WvH' = WvH@WfW, WkH' = WkH@WfW, F2 = WfH@WfW). The
    stage-W fusion conv never runs on device; out_W is materialized once
    (O = x + g2) and feeds all H matmuls. Removes 2 of 8 big matmul
    passes AND the stage-W PSUM->SBUF eviction on the ACT engine.
  - stage-W qkv matmuls in fp8e4 DoubleRow (full 256-contraction per
    pass, 2x PE): only softmax weights E and the gate see the ~4%
    quantization, both of which are insensitive paths.
  - key-path linearity: ctx = Wk @ (sum_r x*E) / S (N=4096 -> 64).
  - q row replicated 128x in its m-tile -> exp(q) lands partition-broadcast.
  - sigmoid via tanh: {exp, tanh, copy} share ONE ACT table set
    (AF.Sigmoid does not -- a table swap costs 1.3us).
  - DVE has ~300ns/op overhead at ~0.38ns/elem/p streaming: elementwise
    ops (u, gate+1, g2, O) run on 1024-column chunk PAIRS; E lives in
    slot 0 of the [128, 3, HW] u tile so E+u reduce in ONE merged tree.
  - y stored bf16 on device, upcast to f32 on host (halves y DMA).
"""

import numpy as np
import ml_dtypes
from contextlib import ExitStack

import concourse.bass as bass
import concourse.bacc as bacc
import concourse.tile as tile
import concourse.mybir as mybir
from concourse.bass_utils import run_bass_kernel_spmd

F32 = mybir.dt.float32
BF16 = mybir.dt.bfloat16
FP8 = mybir.dt.float8e4
AF = mybir.ActivationFunctionType
ALU = mybir.AluOpType
AX = mybir.AxisListType
PM = mybir.MatmulPerfMode
NPBF = ml_dtypes.bfloat16
NPF8 = ml_dtypes.float8_e4m3fn

B, C, H, W = 16, 256, 64, 64
HW = H * W
NCORES = 8
BPC = B // NCORES
KT = 2
CH = 512
NCH = HW // CH
GRP = CH // 64
CP = 2 * CH          # DVE pair width
NP_ = HW // CP       # pairs per stage-batch

_BUILD_CACHE = {}
LAST_RESULTS = None


class _Stage:
    """One attention stage for one batch: p1 (qkv+u), p2 (reduce+ctx), p3."""

    def __init__(self, nc, pools, axis_w, src, stat, wk, bias, src8=None):
        self.nc, self.axis_w = nc, axis_w
        self.src, self.stat, self.wk, self.bias = src, stat, wk, bias
        self.src8 = src8  # callable j -> fp8 [128, KT, CH] chunk (W stage only)
        (self.pbig, self.pgate, self.pchunk, self.pctx, self.pq, self.pv,
         self.phv) = pools

    def p1_alloc(self):
        # slot 0 = E (exp(q)), slots 1:3 = u = x*E  -> one merged reduce tree
        self.u = self.pbig.tile([128, 3, HW], BF16, tag="u")
        self.gate = self.pgate.tile([128, 2, HW], BF16, tag="gate")

    def p1_chunk(self, j):
        nc, stat, bias = self.nc, self.stat, self.bias
        sl = bass.ts(j, CH)
        bv2 = bias.get("bv2")
        ps_q = self.pq.tile([128, CH], F32, tag="q")
        ps_v = self.pv.tile([128, 2 * CH], F32, tag="vf")
        if self.src8 is not None:
            x8 = self.src8(j)
            nc.tensor.matmul(ps_q[:], stat[:, :, 2, :], x8[:], perf_mode=PM.DoubleRow)
            nc.tensor.matmul(ps_v[:, 0:CH], stat[:, :, 0, :], x8[:], perf_mode=PM.DoubleRow)
            nc.tensor.matmul(ps_v[:, CH:], stat[:, :, 1, :], x8[:], perf_mode=PM.DoubleRow)
        else:
            for kt in range(KT):
                st, sp = kt == 0, kt == KT - 1
                rhs = self.src[:, kt, sl]
                nc.tensor.matmul(ps_q[:], stat[:, kt, 2, :], rhs, start=st, stop=sp)
                nc.tensor.matmul(ps_v[:, 0:CH], stat[:, kt, 0, :], rhs, start=st, stop=sp)
                nc.tensor.matmul(ps_v[:, CH:], stat[:, kt, 1, :], rhs, start=st, stop=sp)
        nc.scalar.activation(self.u[:, 0, sl], ps_q[:], AF.Exp, bias=bias["zb"])
        if bv2 is None:
            nc.scalar.activation(
                self.gate[:, :, sl], ps_v[:].rearrange("p (c n) -> p c n", c=2),
                AF.Tanh, scale=0.5,
            )
        else:
            nc.scalar.activation(self.gate[:, 0, sl], ps_v[:, 0:CH], AF.Tanh, bias=bv2[0], scale=0.5)
            nc.scalar.activation(self.gate[:, 1, sl], ps_v[:, CH:], AF.Tanh, bias=bv2[1], scale=0.5)

    def p1_pair(self, jp):
        """u = x*E and gate += 1 over a 1024-column chunk pair."""
        nc = self.nc
        slp = bass.ts(jp, CP)
        eb = self.u[:, 0, slp].unsqueeze(1).broadcast_to([128, 2, CP])
        nc.vector.tensor_tensor(self.u[:, 1:3, slp], self.src[:, :, slp], eb, op=ALU.mult)
        nc.vector.tensor_scalar_add(self.gate[:, :, slp], self.gate[:, :, slp], 1.0)

    def p2_tree_ops(self):
        """Emit the reduce tree as a list of thunks (DVE filler ops)."""
        nc, pctx, phv = self.nc, self.pctx, self.phv
        t3 = phv.tile([128, 3, 2048], BF16, tag="t3")
        SX = pctx.tile([128, 3, 64], F32, tag="SX")
        self.SX = SX
        ops = []
        if self.axis_w:
            # reduce over w (inner 64): [3, 64 groups, r] halving + TR
            v4 = self.u[:].rearrange("p c (a r) -> p c a r", r=64)
            hv = t3[:].rearrange("p c (a r) -> p c a r", r=32)
            ops.append(lambda: nc.vector.tensor_tensor(hv[:, :, :, :], v4[:, :, :, 0:32], v4[:, :, :, 32:64], op=ALU.add))
            ops.append(lambda: nc.vector.tensor_tensor(hv[:, :, :, 0:16], hv[:, :, :, 0:16], hv[:, :, :, 16:32], op=ALU.add))
            ops.append(lambda: nc.vector.tensor_tensor(hv[:, :, :, 0:8], hv[:, :, :, 0:8], hv[:, :, :, 8:16], op=ALU.add))
            ops.append(lambda: nc.vector.tensor_reduce(SX[:], hv[:, :, :, 0:8], axis=AX.X, op=ALU.add))
        else:
            # reduce over h (outer): contiguous halving tree on [3, n]
            ops.append(lambda: nc.vector.tensor_tensor(t3[:, :, :], self.u[:, :, 0:2048], self.u[:, :, 2048:4096], op=ALU.add))
            def lvl(n):
                return lambda: nc.vector.tensor_tensor(t3[:, :, 0:n], t3[:, :, 0:n], t3[:, :, n:2 * n], op=ALU.add)
            n = 1024
            while n >= 128:
                ops.append(lvl(n))
                n //= 2
            ops.append(lambda: nc.vector.tensor_tensor(SX[:], t3[:, :, 0:64], t3[:, :, 64:128], op=ALU.add))
        return ops

    def p2_ctx(self):
        nc, pctx = self.nc, self.pctx
        SX = self.SX
        R = pctx.tile([128, 64], F32, tag="R")
        nc.vector.reciprocal(R[:], SX[:, 0, :])
        xn = pctx.tile([128, 2, 64], BF16, tag="xn")
        rb = R[:].unsqueeze(1).broadcast_to([128, 2, 64])
        nc.vector.tensor_tensor(xn[:], SX[:, 1:3, :], rb, op=ALU.mult)

        bk2 = self.bias.get("bk2")
        ctx_t = pctx.tile([128, 2, 64], BF16, tag="ctx")
        for mt in range(2):
            ps_c = self.pq.tile([128, 64], F32, tag="q")
            for ct in range(2):
                nc.tensor.matmul(ps_c[:], self.wk[:, ct, mt, :], xn[:, ct, :],
                                 start=ct == 0, stop=ct == 1)
            if bk2 is None:
                nc.vector.tensor_scalar_mul(ctx_t[:, mt, :], ps_c[:], 0.5)
            else:
                nc.vector.tensor_scalar(ctx_t[:, mt, :], ps_c[:], 0.5, bk2[mt],
                                        op0=ALU.mult, op1=ALU.add)
        self.ctx_t = ctx_t

    def g2_pair(self, jp):
        """g2 = gate1 * ctx_broadcast for a chunk pair -> [128, 2, 2*GRP, 64]."""
        nc = self.nc
        g2 = self.pchunk.tile([128, 2, 2 * GRP, 64], BF16, tag="g2")
        gv = self.gate[:, :, bass.ts(jp, CP)].rearrange("p c (a r) -> p c a r", r=64)
        if self.axis_w:
            cb = self.ctx_t[:, :, bass.ts(jp, 2 * GRP)].unsqueeze(3).broadcast_to([128, 2, 2 * GRP, 64])
        else:
            cb = self.ctx_t[:].unsqueeze(2).broadcast_to([128, 2, 2 * GRP, 64])
        nc.vector.tensor_tensor(g2[:], gv, cb, op=ALU.mult)
        return g2


def _build(flags):
    bvW0, bkW0, bvH0, bkH0, byH0 = flags
    nc = bacc.Bacc(trn_type="TRN2", target_bir_lowering=False, debug=False)

    x_d = nc.dram_tensor("x", [BPC, C, HW], BF16, kind="ExternalInput").ap()
    statW_d = nc.dram_tensor("statW", [128, KT, 3, 128], BF16, kind="ExternalInput").ap()
    statH_d = nc.dram_tensor("statH", [128, KT, 3, 128], BF16, kind="ExternalInput").ap()
    wkW_d = nc.dram_tensor("wkW", [128, KT, 2, 128], BF16, kind="ExternalInput").ap()
    wkH_d = nc.dram_tensor("wkH", [128, KT, 2, 128], BF16, kind="ExternalInput").ap()
    f2_d = nc.dram_tensor("f2", [128, KT, 2, 128], BF16, kind="ExternalInput").ap()
    fg_d = nc.dram_tensor("fg", [128, KT, 2, 128], BF16, kind="ExternalInput").ap()
    bias_d = nc.dram_tensor("biases", [5, 2, 128], F32, kind="ExternalInput").ap()
    y_d = nc.dram_tensor("y", [BPC, C, HW], BF16, kind="ExternalOutput").ap()

    with tile.TileContext(nc) as tc, ExitStack() as ctx:
        wp = ctx.enter_context(tc.tile_pool(name="weights", bufs=1))
        xp = ctx.enter_context(tc.tile_pool(name="x", bufs=2))
        op_ = ctx.enter_context(tc.tile_pool(name="O", bufs=2))
        pbig = ctx.enter_context(tc.tile_pool(name="big", bufs=2))
        pgate = ctx.enter_context(tc.tile_pool(name="gate", bufs=3))
        pchunk = ctx.enter_context(tc.tile_pool(name="chunk", bufs=2))
        pctx = ctx.enter_context(tc.tile_pool(name="ctx", bufs=3))
        phv = ctx.enter_context(tc.tile_pool(name="hv", bufs=1))
        yp = ctx.enter_context(tc.tile_pool(name="yev", bufs=2))
        pq = ctx.enter_context(tc.tile_pool(name="psq", bufs=2, space="PSUM"))
        pvf = ctx.enter_context(tc.tile_pool(name="psvf", bufs=3, space="PSUM"))
        pools = (pbig, pgate, pchunk, pctx, pq, pvf, phv)

        def wload(name, dram, shape, dt):
            t = wp.tile(shape, dt, tag=name)
            nc.scalar.dma_start(t[:], dram[:])
            return t

        statW = wload("statW", statW_d, [128, KT, 3, 128], BF16)
        statH = wload("statH", statH_d, [128, KT, 3, 128], BF16)
        wkW = wload("wkW", wkW_d, [128, KT, 2, 128], BF16)
        wkH = wload("wkH", wkH_d, [128, KT, 2, 128], BF16)
        f2 = wload("f2", f2_d, [128, KT, 2, 128], BF16)
        fg = wload("fg", fg_d, [128, KT, 2, 128], BF16)

        bias_sb = wp.tile([128, 5, 2], F32, tag="biases")
        nc.scalar.dma_start(bias_sb[:], bias_d[:].transpose([2, 0, 1]))
        zb = wp.tile([128, 1], F32, tag="zb")
        nc.vector.memset(zb[:], 0.0)

        def bap(i, ct):
            return bias_sb[:, i, ct].unsqueeze(1)

        biasW = {
            "bv2": None if bvW0 else [bap(0, ct) for ct in range(2)],
            "bk2": None if bkW0 else [bap(1, ct) for ct in range(2)],
            "zb": zb[:],
        }
        biasH = {
            "bv2": None if bvH0 else [bap(2, ct) for ct in range(2)],
            "bk2": None if bkH0 else [bap(3, ct) for ct in range(2)],
            "zb": zb[:],
        }

        def load_x(b):
            # chunk-major, kt split across two DMA queues: first compute
            # chunk is ready after ~2 small transfers, not 9
            x = xp.tile([128, KT, HW], BF16, tag="x")
            for j in range(NCH):
                for kt in range(KT):
                    nc.sync.dma_start(x[:, kt, bass.ts(j, CH)],
                                      x_d[b, bass.ts(kt, 128), bass.ts(j, CH)])
            return x

        def make_O(b):
            O_t = op_.tile([128, KT, HW], BF16, tag="O")
            return O_t

        # stage-W p3 pair: O = x + gate1*ctxb  (no matmul; feeds all H work)
        def w_p3_pair(st, x, O, jp):
            g2 = st.g2_pair(jp)
            g2f = g2[:].rearrange("p c a r -> p c (a r)")
            nc.vector.tensor_tensor(O[:, :, bass.ts(jp, CP)], x[:, :, bass.ts(jp, CP)], g2f, op=ALU.add)

        # stage-H p3: y = F2@O + FG@g2H per chunk, evict bf16
        def h_p3_chunk(st, O, g2p, b, j):
            ps_f = pvf.tile([128, 2 * CH], F32, tag="vf")
            sl = bass.ts(j, CH)
            ghalf = g2p[:, :, bass.ts(j % 2, GRP), :]
            for mt in range(2):
                half = ps_f[:, bass.ts(mt, CH)]
                nc.tensor.matmul(half, f2[:, 0, mt, :], O[:, 0, sl], start=True, stop=False)
                nc.tensor.matmul(half, f2[:, 1, mt, :], O[:, 1, sl], start=False, stop=False)
                nc.tensor.matmul(half, fg[:, 0, mt, :], ghalf[:, 0].rearrange("p a r -> p (a r)"), start=False, stop=False)
                nc.tensor.matmul(half, fg[:, 1, mt, :], ghalf[:, 1].rearrange("p a r -> p (a r)"), start=False, stop=True)
            y_t = yp.tile([128, 2, CH], BF16, tag="y")
            if byH0:
                nc.scalar.activation(y_t[:], ps_f[:].rearrange("p (c n) -> p c n", c=2), AF.Copy)
            else:
                for ct in range(2):
                    nc.scalar.activation(y_t[:, ct, :], ps_f[:, bass.ts(ct, CH)],
                                         AF.Identity, bias=bap(4, ct))
            nc.sync.dma_start(
                y_d[b].rearrange("(c p) n -> p c n", p=128)[:, :, sl], y_t[:])

        x0 = load_x(0)
        x1 = load_x(1)
        w0 = _Stage(nc, pools, True, x0[:], statW, wkW, biasW)
        w1 = _Stage(nc, pools, True, x1[:], statW, wkW, biasW)

        def run_p1(st):
            # lag the u/gate pairs one pair behind the PSUM chunks so the
            # DVE never waits on the PE->ACT roundtrip of its own pair
            st.p1_alloc()
            for j in range(NCH):
                st.p1_chunk(j)
                if j % 2 == 1 and j >= 3:
                    st.p1_pair((j - 2) // 2)
            st.p1_pair(NP_ - 1)

        run_p1(w0)
        run_p1(w1)
        w0.p2_tree_ops_run = [f() for f in w0.p2_tree_ops()]
        w0.p2_ctx()

        # [W-p3 + H-p1] loops; the OTHER batch's reduce tree is interleaved
        # as DVE filler so the vector engine never idles on cross-engine deps
        def p3p1_loop(wst, xt, Ot, hst, filler):
            hst.p1_alloc()
            fill = list(filler)
            for jp in range(NP_):
                w_p3_pair(wst, xt, Ot, jp)
                if fill:
                    fill.pop(0)()
                hst.p1_chunk(2 * jp)
                hst.p1_chunk(2 * jp + 1)
                if jp >= 1:
                    hst.p1_pair(jp - 1)
                if fill:
                    fill.pop(0)()
            for f in fill:
                f()
            hst.p1_pair(NP_ - 1)

        O0 = make_O(0)
        h0 = _Stage(nc, pools, False, O0[:], statH, wkH, biasH)
        p3p1_loop(w0, x0, O0, h0, w1.p2_tree_ops())
        w1.p2_ctx()

        O1 = make_O(1)
        h1 = _Stage(nc, pools, False, O1[:], statH, wkH, biasH)
        p3p1_loop(w1, x1, O1, h1, h0.p2_tree_ops())
        h0.p2_ctx()

        # [H-p3] loops; the other H batch's tree fills the DVE
        h1_tree = h1.p2_tree_ops()
        for jp in range(NP_):
            g2p = h0.g2_pair(jp)
            if h1_tree:
                h1_tree.pop(0)()
            h_p3_chunk(h0, O0, g2p, 0, 2 * jp)
            h_p3_chunk(h0, O0, g2p, 0, 2 * jp + 1)
            if h1_tree:
                h1_tree.pop(0)()
        for f in h1_tree:
            f()
        h1.p2_ctx()
        for jp in range(NP_):
            g2p = h1.g2_pair(jp)
            h_p3_chunk(h1, O1, g2p, 1, 2 * jp)
            h_p3_chunk(h1, O1, g2p, 1, 2 * jp + 1)

    nc.compile()
    return nc


def _to_stat(wq, wv):
    """[q replicated; v] -> lhsT layout [128, KT, 3, 128]."""
    stat = np.empty((128, KT, 3, 128), np.float32)
    for kt in range(KT):
        cs = slice(kt * 128, (kt + 1) * 128)
        stat[:, kt, 0, :] = wv[0:128, cs].T
        stat[:, kt, 1, :] = wv[128:256, cs].T
        stat[:, kt, 2, :] = np.repeat(wq[cs][:, None], 128, axis=1)
    return stat


def _to_lhsT(w):
    """[256out, 256in] -> [128, KT, 2, 128] (k-tile, m-tile)."""
    t = np.empty((128, KT, 2, 128), np.float32)
    for kt in range(KT):
        cs = slice(kt * 128, (kt + 1) * 128)
        t[:, kt, 0, :] = w[0:128, cs].T
        t[:, kt, 1, :] = w[128:256, cs].T
    return t


def kernel(x, qkvW_w, qkvW_b, qkvH_w, qkvH_b, fusW_w, fusW_b, fusH_w, fusH_b):
    global LAST_RESULTS
    x = np.asarray(x, np.float32)
    f64 = lambda a: np.asarray(a, np.float64)
    qkvW_w, qkvW_b = f64(qkvW_w), f64(qkvW_b)
    qkvH_w, qkvH_b = f64(qkvH_w), f64(qkvH_b)
    fusW_w, fusW_b = f64(fusW_w), f64(fusW_b)
    fusH_w, fusH_b = f64(fusH_w), f64(fusH_b)

    wqW, wkW_m, wvW = qkvW_w[0], qkvW_w[1:1 + C], qkvW_w[1 + C:]
    wqH, wkH_m, wvH = qkvH_w[0], qkvH_w[1:1 + C], qkvH_w[1 + C:]

    # collapse WfW into stage-H weights (x_w = WfW @ O + bfW)
    wqHp = wqH @ fusW_w
    wvHp = wvH @ fusW_w
    wkHp = wkH_m @ fusW_w
    F2 = fusH_w @ fusW_w

    statW = _to_stat(wqW.astype(np.float32), wvW.astype(np.float32))
    statH = _to_stat(wqHp.astype(np.float32), wvHp.astype(np.float32))
    wkWl = _to_lhsT(wkW_m.astype(np.float32))
    wkHl = _to_lhsT(wkHp.astype(np.float32))
    f2l = _to_lhsT(F2.astype(np.float32))
    fgl = _to_lhsT(fusH_w.astype(np.float32))

    # tanh-gate folding: ACT computes tanh(0.5*v + 0.5*bv); ctx is scaled by
    # 0.5 on device, so the ctx bias constant also carries the 0.5.
    bvW = qkvW_b[1 + C:]
    bkW = qkvW_b[1:1 + C]
    bvHp = wvH @ fusW_b + qkvH_b[1 + C:]
    bkHp = wkH_m @ fusW_b + qkvH_b[1:1 + C]   # sum_h softmax = 1 -> adds to ctx
    byH = fusH_w @ fusW_b + fusH_b
    biases = np.stack([
        0.5 * bvW.reshape(2, 128), 0.5 * bkW.reshape(2, 128),
        0.5 * bvHp.reshape(2, 128), 0.5 * bkHp.reshape(2, 128),
        byH.reshape(2, 128),
    ]).astype(np.float32)

    flags = (
        not bvW.any(), not bkW.any(), not bvHp.any(), not bkHp.any(), not byH.any(),
    )
    if flags not in _BUILD_CACHE:
        _BUILD_CACHE[flags] = _build(flags)
    nc = _BUILD_CACHE[flags]

    tobf = lambda a: np.ascontiguousarray(a.astype(NPBF))
    xbf = np.ascontiguousarray(x.reshape(B, C, HW).astype(NPBF))
    in_maps = []
    for core in range(NCORES):
        in_maps.append({
            "x": xbf[core * BPC: (core + 1) * BPC],
            "statW": tobf(statW), "statH": tobf(statH),
            "wkW": tobf(wkWl), "wkH": tobf(wkHl),
            "f2": tobf(f2l), "fg": tobf(fgl),
            "biases": biases,
        })

    res = run_bass_kernel_spmd(nc, in_maps, list(range(NCORES)))
    LAST_RESULTS = res
    y = np.concatenate([r["y"] for r in res.results], axis=0)
    return y.astype(np.float32).reshape(B, C, H, W)


# revision 16
# speedup vs baseline: 1.1743x; 1.1743x over previous
"""Trainium2 Bass kernel for nn_DualAxisAggAttn (dual-axis aggregation attention).

Reference semantics per batch image x[C=256, H=64, W=64], twice (W axis then H axis):
  qkv = conv1x1(x) -> {q:[1], k:[C], v:[C]};  s = softmax_axis(q)
  ctx[c,a] = sum_r k*s;  out = x + sigmoid(v) * ctx_bcast;  y = conv1x1(out)

Distribution: data-parallel over batch (16 images -> 2 per NeuronCore x 8 cores).

Structure (v4):
  - STAGE COLLAPSE: every stage-H op on x_w = WfW @ out_W is linear in
    channels, so WfW is folded into stage-H's weights on the HOST
    (qH' = qH@WfW, WvH' = WvH@WfW, WkH' = WkH@WfW, F2 = WfH@WfW). The
    stage-W fusion conv never runs on device; out_W is materialized once
    (O = x + g2) and feeds all H matmuls. Removes 2 of 8 big matmul
    passes AND the stage-W PSUM->SBUF eviction on the ACT engine.
  - stage-W qkv matmuls in fp8e4 DoubleRow (full 256-contraction per
    pass, 2x PE): only softmax weights E and the gate see the ~4%
    quantization, both of which are insensitive paths.
  - key-path linearity: ctx = Wk @ (sum_r x*E) / S (N=4096 -> 64).
  - q row replicated 128x in its m-tile -> exp(q) lands partition-broadcast.
  - sigmoid via tanh: {exp, tanh, copy} share ONE ACT table set
    (AF.Sigmoid does not -- a table swap costs 1.3us).
  - DVE has ~300ns/op overhead at ~0.38ns/elem/p streaming: elementwise
    ops (u, gate+1, g2, O) run on 1024-column chunk PAIRS; E lives in
    slot 0 of the [128, 3, HW] u tile so E+u reduce in ONE merged tree.
  - y stored bf16 on device, upcast to f32 on host (halves y DMA).
"""

import numpy as np
import ml_dtypes
from contextlib import ExitStack

import concourse.bass as bass
import concourse.bacc as bacc
import concourse.tile as tile
import concourse.mybir as mybir
from concourse.bass_utils import run_bass_kernel_spmd

F32 = mybir.dt.float32
BF16 = mybir.dt.bfloat16
FP8 = mybir.dt.float8e4
AF = mybir.ActivationFunctionType
ALU = mybir.AluOpType
AX = mybir.AxisListType
PM = mybir.MatmulPerfMode
NPBF = ml_dtypes.bfloat16
NPF8 = ml_dtypes.float8_e4m3fn

B, C, H, W = 16, 256, 64, 64
HW = H * W
NCORES = 8
BPC = B // NCORES
KT = 2
CH = 512
NCH = HW // CH
GRP = CH // 64
CP = 2 * CH          # DVE pair width
NP_ = HW // CP       # pairs per stage-batch

_BUILD_CACHE = {}
LAST_RESULTS = None


class _Stage:
    """One attention stage for one batch: p1 (qkv+u), p2 (reduce+ctx), p3."""

    def __init__(self, nc, pools, axis_w, src, stat, wk, bias, src8=None):
        self.nc, self.axis_w = nc, axis_w
        self.src, self.stat, self.wk, self.bias = src, stat, wk, bias
        self.src8 = src8  # callable j -> fp8 [128, KT, CH] chunk (W stage only)
        (self.pbig, self.pgate, self.pchunk, self.pctx, self.pq, self.pv,
         self.phv) = pools

    def p1_alloc(self):
        # slot 0 = E (exp(q)), slots 1:3 = u = x*E  -> one merged reduce tree
        self.u = self.pbig.tile([128, 3, HW], BF16, tag="u")
        self.gate = self.pgate.tile([128, 2, HW], BF16, tag="gate")
        self._pend_tanh = {}

    def p1_chunk(self, j):
        nc, stat, bias = self.nc, self.stat, self.bias
        sl = bass.ts(j, CH)
        bv2 = bias.get("bv2")
        ps_q = self.pq.tile([128, CH], F32, tag="q")
        ps_v = self.pv.tile([128, 2 * CH], F32, tag="vf")
        if self.src8 is not None:
            x8 = self.src8(j)
            nc.tensor.matmul(ps_q[:], stat[:, :, 2, :], x8[:], perf_mode=PM.DoubleRow)
            nc.tensor.matmul(ps_v[:, 0:CH], stat[:, :, 0, :], x8[:], perf_mode=PM.DoubleRow)
            nc.tensor.matmul(ps_v[:, CH:], stat[:, :, 1, :], x8[:], perf_mode=PM.DoubleRow)
        else:
            for kt in range(KT):
                st, sp = kt == 0, kt == KT - 1
                rhs = self.src[:, kt, sl]
                nc.tensor.matmul(ps_q[:], stat[:, kt, 2, :], rhs, start=st, stop=sp)
                nc.tensor.matmul(ps_v[:, 0:CH], stat[:, kt, 0, :], rhs, start=st, stop=sp)
                nc.tensor.matmul(ps_v[:, CH:], stat[:, kt, 1, :], rhs, start=st, stop=sp)
        nc.scalar.activation(self.u[:, 0, sl], ps_q[:], AF.Exp, bias=bias["zb"])
        self._pend_tanh[j] = ps_v

    def p1_tanh(self, j):
        """Deferred gate tanh: exp leads by a chunk so u never waits on it."""
        nc, bias = self.nc, self.bias
        sl = bass.ts(j, CH)
        ps_v = self._pend_tanh.pop(j)
        bv2 = bias.get("bv2")
        if bv2 is None:
            nc.scalar.activation(
                self.gate[:, :, sl], ps_v[:].rearrange("p (c n) -> p c n", c=2),
                AF.Tanh, scale=0.5,
            )
        else:
            nc.scalar.activation(self.gate[:, 0, sl], ps_v[:, 0:CH], AF.Tanh, bias=bv2[0], scale=0.5)
            nc.scalar.activation(self.gate[:, 1, sl], ps_v[:, CH:], AF.Tanh, bias=bv2[1], scale=0.5)

    def p1_pair(self, jp):
        """u = x*E over a 1024-column chunk pair."""
        nc = self.nc
        slp = bass.ts(jp, CP)
        eb = self.u[:, 0, slp].unsqueeze(1).broadcast_to([128, 2, CP])
        nc.vector.tensor_tensor(self.u[:, 1:3, slp], self.src[:, :, slp], eb, op=ALU.mult)

    def p1_ts(self, jp):
        nc = self.nc
        slp = bass.ts(jp, CP)
        nc.vector.tensor_scalar_add(self.gate[:, :, slp], self.gate[:, :, slp], 1.0)

    def p2_tree_ops(self):
        """Emit the reduce tree as a list of thunks (DVE filler ops)."""
        nc, pctx, phv = self.nc, self.pctx, self.phv
        t3 = phv.tile([128, 3, 2048], BF16, tag="t3")
        SX = pctx.tile([128, 3, 64], F32, tag="SX")
        self.SX = SX
        ops = []
        if self.axis_w:
            # reduce over w (inner 64): [3, 64 groups, r] halving + TR
            v4 = self.u[:].rearrange("p c (a r) -> p c a r", r=64)
            hv = t3[:].rearrange("p c (a r) -> p c a r", r=32)
            ops.append(lambda: nc.vector.tensor_tensor(hv[:, :, :, :], v4[:, :, :, 0:32], v4[:, :, :, 32:64], op=ALU.add))
            ops.append(lambda: nc.vector.tensor_tensor(hv[:, :, :, 0:16], hv[:, :, :, 0:16], hv[:, :, :, 16:32], op=ALU.add))
            ops.append(lambda: nc.vector.tensor_tensor(hv[:, :, :, 0:8], hv[:, :, :, 0:8], hv[:, :, :, 8:16], op=ALU.add))
            ops.append(lambda: nc.vector.tensor_reduce(SX[:], hv[:, :, :, 0:8], axis=AX.X, op=ALU.add))
        else:
            # reduce over h (outer): contiguous halving tree on [3, n]
            ops.append(lambda: nc.vector.tensor_tensor(t3[:, :, :], self.u[:, :, 0:2048], self.u[:, :, 2048:4096], op=ALU.add))
            def lvl(n):
                return lambda: nc.vector.tensor_tensor(t3[:, :, 0:n], t3[:, :, 0:n], t3[:, :, n:2 * n], op=ALU.add)
            n = 1024
            while n >= 128:
                ops.append(lvl(n))
                n //= 2
            ops.append(lambda: nc.vector.tensor_tensor(SX[:], t3[:, :, 0:64], t3[:, :, 64:128], op=ALU.add))
        return ops

    def p2_ctx(self):
        nc, pctx = self.nc, self.pctx
        SX = self.SX
        R = pctx.tile([128, 64], F32, tag="R")
        nc.vector.reciprocal(R[:], SX[:, 0, :])
        xn = pctx.tile([128, 2, 64], BF16, tag="xn")
        rb = R[:].unsqueeze(1).broadcast_to([128, 2, 64])
        nc.vector.tensor_tensor(xn[:], SX[:, 1:3, :], rb, op=ALU.mult)

        bk2 = self.bias.get("bk2")
        ctx_t = pctx.tile([128, 2, 64], BF16, tag="ctx")
        for mt in range(2):
            ps_c = self.pq.tile([128, 64], F32, tag="q")
            for ct in range(2):
                nc.tensor.matmul(ps_c[:], self.wk[:, ct, mt, :], xn[:, ct, :],
                                 start=ct == 0, stop=ct == 1)
            if bk2 is None:
                nc.vector.tensor_scalar_mul(ctx_t[:, mt, :], ps_c[:], 0.5)
            else:
                nc.vector.tensor_scalar(ctx_t[:, mt, :], ps_c[:], 0.5, bk2[mt],
                                        op0=ALU.mult, op1=ALU.add)
        self.ctx_t = ctx_t

    def g2_pair(self, jp):
        """g2 = gate1 * ctx_broadcast for a chunk pair -> [128, 2, 2*GRP, 64]."""
        nc = self.nc
        g2 = self.pchunk.tile([128, 2, 2 * GRP, 64], BF16, tag="g2")
        gv = self.gate[:, :, bass.ts(jp, CP)].rearrange("p c (a r) -> p c a r", r=64)
        if self.axis_w:
            cb = self.ctx_t[:, :, bass.ts(jp, 2 * GRP)].unsqueeze(3).broadcast_to([128, 2, 2 * GRP, 64])
        else:
            cb = self.ctx_t[:].unsqueeze(2).broadcast_to([128, 2, 2 * GRP, 64])
        nc.vector.tensor_tensor(g2[:], gv, cb, op=ALU.mult)
        return g2


def _build(flags):
    bvW0, bkW0, bvH0, bkH0, byH0 = flags
    nc = bacc.Bacc(trn_type="TRN2", target_bir_lowering=False, debug=False)

    x_d = nc.dram_tensor("x", [BPC, C, HW], BF16, kind="ExternalInput").ap()
    statW_d = nc.dram_tensor("statW", [128, KT, 3, 128], BF16, kind="ExternalInput").ap()
    statH_d = nc.dram_tensor("statH", [128, KT, 3, 128], BF16, kind="ExternalInput").ap()
    wkW_d = nc.dram_tensor("wkW", [128, KT, 2, 128], BF16, kind="ExternalInput").ap()
    wkH_d = nc.dram_tensor("wkH", [128, KT, 2, 128], BF16, kind="ExternalInput").ap()
    f2_d = nc.dram_tensor("f2", [128, KT, 2, 128], BF16, kind="ExternalInput").ap()
    fg_d = nc.dram_tensor("fg", [128, KT, 2, 128], BF16, kind="ExternalInput").ap()
    bias_d = nc.dram_tensor("biases", [5, 2, 128], F32, kind="ExternalInput").ap()
    y_d = nc.dram_tensor("y", [BPC, C, HW], BF16, kind="ExternalOutput").ap()

    with tile.TileContext(nc) as tc, ExitStack() as ctx:
        wp = ctx.enter_context(tc.tile_pool(name="weights", bufs=1))
        xp = ctx.enter_context(tc.tile_pool(name="x", bufs=2))
        op_ = ctx.enter_context(tc.tile_pool(name="O", bufs=2))
        pbig = ctx.enter_context(tc.tile_pool(name="big", bufs=2))
        pgate = ctx.enter_context(tc.tile_pool(name="gate", bufs=3))
        pchunk = ctx.enter_context(tc.tile_pool(name="chunk", bufs=2))
        pctx = ctx.enter_context(tc.tile_pool(name="ctx", bufs=3))
        phv = ctx.enter_context(tc.tile_pool(name="hv", bufs=1))
        yp = ctx.enter_context(tc.tile_pool(name="yev", bufs=2))
        pq = ctx.enter_context(tc.tile_pool(name="psq", bufs=2, space="PSUM"))
        pvf = ctx.enter_context(tc.tile_pool(name="psvf", bufs=3, space="PSUM"))
        pools = (pbig, pgate, pchunk, pctx, pq, pvf, phv)

        def wload(name, dram, shape, dt):
            t = wp.tile(shape, dt, tag=name)
            nc.scalar.dma_start(t[:], dram[:])
            return t

        statW = wload("statW", statW_d, [128, KT, 3, 128], BF16)
        statH = wload("statH", statH_d, [128, KT, 3, 128], BF16)
        wkW = wload("wkW", wkW_d, [128, KT, 2, 128], BF16)
        wkH = wload("wkH", wkH_d, [128, KT, 2, 128], BF16)
        f2 = wload("f2", f2_d, [128, KT, 2, 128], BF16)
        fg = wload("fg", fg_d, [128, KT, 2, 128], BF16)

        bias_sb = wp.tile([128, 5, 2], F32, tag="biases")
        nc.scalar.dma_start(bias_sb[:], bias_d[:].transpose([2, 0, 1]))
        zb = wp.tile([128, 1], F32, tag="zb")
        nc.vector.memset(zb[:], 0.0)

        def bap(i, ct):
            return bias_sb[:, i, ct].unsqueeze(1)

        biasW = {
            "bv2": None if bvW0 else [bap(0, ct) for ct in range(2)],
            "bk2": None if bkW0 else [bap(1, ct) for ct in range(2)],
            "zb": zb[:],
        }
        biasH = {
            "bv2": None if bvH0 else [bap(2, ct) for ct in range(2)],
            "bk2": None if bkH0 else [bap(3, ct) for ct in range(2)],
            "zb": zb[:],
        }

        def load_x(b):
            # chunk-major, kt split across two DMA queues: first compute
            # chunk is ready after ~2 small transfers, not 9
            x = xp.tile([128, KT, HW], BF16, tag="x")
            for j in range(NCH):
                for kt in range(KT):
                    nc.sync.dma_start(x[:, kt, bass.ts(j, CH)],
                                      x_d[b, bass.ts(kt, 128), bass.ts(j, CH)])
            return x

        def make_O(b):
            O_t = op_.tile([128, KT, HW], BF16, tag="O")
            return O_t

        # stage-W p3 pair: O = x + gate1*ctxb  (no matmul; feeds all H work)
        def w_p3_pair(st, x, O, jp):
            g2 = st.g2_pair(jp)
            g2f = g2[:].rearrange("p c a r -> p c (a r)")
            nc.vector.tensor_tensor(O[:, :, bass.ts(jp, CP)], x[:, :, bass.ts(jp, CP)], g2f, op=ALU.add)

        # stage-H p3: y = F2@O + FG@g2H per chunk, evict bf16
        def h_p3_chunk(st, O, g2p, b, j):
            ps_f = pvf.tile([128, 2 * CH], F32, tag="vf")
            sl = bass.ts(j, CH)
            ghalf = g2p[:, :, bass.ts(j % 2, GRP), :]
            for mt in range(2):
                half = ps_f[:, bass.ts(mt, CH)]
                nc.tensor.matmul(half, f2[:, 0, mt, :], O[:, 0, sl], start=True, stop=False)
                nc.tensor.matmul(half, f2[:, 1, mt, :], O[:, 1, sl], start=False, stop=False)
                nc.tensor.matmul(half, fg[:, 0, mt, :], ghalf[:, 0].rearrange("p a r -> p (a r)"), start=False, stop=False)
                nc.tensor.matmul(half, fg[:, 1, mt, :], ghalf[:, 1].rearrange("p a r -> p (a r)"), start=False, stop=True)
            y_t = yp.tile([128, 2, CH], BF16, tag="y")
            if byH0:
                nc.scalar.activation(y_t[:], ps_f[:].rearrange("p (c n) -> p c n", c=2), AF.Copy)
            else:
                for ct in range(2):
                    nc.scalar.activation(y_t[:, ct, :], ps_f[:, bass.ts(ct, CH)],
                                         AF.Identity, bias=bap(4, ct))
            nc.sync.dma_start(
                y_d[b].rearrange("(c p) n -> p c n", p=128)[:, :, sl], y_t[:])

        x0 = load_x(0)
        x1 = load_x(1)
        w0 = _Stage(nc, pools, True, x0[:], statW, wkW, biasW)
        w1 = _Stage(nc, pools, True, x1[:], statW, wkW, biasW)

        def run_p1(st):
            # exp leads tanh by one chunk; u-pairs lag one pair behind the
            # PSUM chunks so the DVE never waits on the PE->ACT roundtrip
            st.p1_alloc()
            for j in range(NCH):
                st.p1_chunk(j)
                if j >= 1:
                    st.p1_tanh(j - 1)
                if j % 2 == 1 and j >= 3:
                    st.p1_pair((j - 2) // 2)
                    st.p1_ts((j - 2) // 2)
            st.p1_tanh(NCH - 1)
            st.p1_pair(NP_ - 1)
            st.p1_ts(NP_ - 1)

        def run_trees(st):
            for f in st.p2_tree_ops():
                f()
            st.p2_ctx()

        def p3p1_loop(wst, xt, Ot, hst):
            hst.p1_alloc()
            for jp in range(NP_):
                w_p3_pair(wst, xt, Ot, jp)
                hst.p1_chunk(2 * jp)
                hst.p1_chunk(2 * jp + 1)
                if 2 * jp >= 1:
                    hst.p1_tanh(2 * jp - 1)
                hst.p1_tanh(2 * jp)
                if jp >= 1:
                    hst.p1_pair(jp - 1)
                    hst.p1_ts(jp - 1)
            hst.p1_tanh(NCH - 1)
            hst.p1_pair(NP_ - 1)
            hst.p1_ts(NP_ - 1)

        def h_p3_loop(hst, Ot, b):
            for jp in range(NP_):
                g2p = hst.g2_pair(jp)
                h_p3_chunk(hst, Ot, g2p, b, 2 * jp)
                h_p3_chunk(hst, Ot, g2p, b, 2 * jp + 1)

        run_p1(w0)
        run_p1(w1)
        run_trees(w0)

        O0 = make_O(0)
        h0 = _Stage(nc, pools, False, O0[:], statH, wkH, biasH)
        p3p1_loop(w0, x0, O0, h0)

        run_trees(h0)
        run_trees(w1)
        # h0's fusion matmuls+evicts now overlap batch-1's DVE-heavy loop
        h_p3_loop(h0, O0, 0)

        O1 = make_O(1)
        h1 = _Stage(nc, pools, False, O1[:], statH, wkH, biasH)
        p3p1_loop(w1, x1, O1, h1)

        run_trees(h1)
        h_p3_loop(h1, O1, 1)

    nc.compile()
    return nc


def _to_stat(wq, wv):
    """[q replicated; v] -> lhsT layout [128, KT, 3, 128]."""
    stat = np.empty((128, KT, 3, 128), np.float32)
    for kt in range(KT):
        cs = slice(kt * 128, (kt + 1) * 128)
        stat[:, kt, 0, :] = wv[0:128, cs].T
        stat[:, kt, 1, :] = wv[128:256, cs].T
        stat[:, kt, 2, :] = np.repeat(wq[cs][:, None], 128, axis=1)
    return stat


def _to_lhsT(w):
    """[256out, 256in] -> [128, KT, 2, 128] (k-tile, m-tile)."""
    t = np.empty((128, KT, 2, 128), np.float32)
    for kt in range(KT):
        cs = slice(kt * 128, (kt + 1) * 128)
        t[:, kt, 0, :] = w[0:128, cs].T
        t[:, kt, 1, :] = w[128:256, cs].T
    return t


def kernel(x, qkvW_w, qkvW_b, qkvH_w, qkvH_b, fusW_w, fusW_b, fusH_w, fusH_b):
    global LAST_RESULTS
    x = np.asarray(x, np.float32)
    f64 = lambda a: np.asarray(a, np.float64)
    qkvW_w, qkvW_b = f64(qkvW_w), f64(qkvW_b)
    qkvH_w, qkvH_b = f64(qkvH_w), f64(qkvH_b)
    fusW_w, fusW_b = f64(fusW_w), f64(fusW_b)
    fusH_w, fusH_b = f64(fusH_w), f64(fusH_b)

    wqW, wkW_m, wvW = qkvW_w[0], qkvW_w[1:1 + C], qkvW_w[1 + C:]
    wqH, wkH_m, wvH = qkvH_w[0], qkvH_w[1:1 + C], qkvH_w[1 + C:]

    # collapse WfW into stage-H weights (x_w = WfW @ O + bfW)
    wqHp = wqH @ fusW_w
    wvHp = wvH @ fusW_w
    wkHp = wkH_m @ fusW_w
    F2 = fusH_w @ fusW_w

    statW = _to_stat(wqW.astype(np.float32), wvW.astype(np.float32))
    statH = _to_stat(wqHp.astype(np.float32), wvHp.astype(np.float32))
    wkWl = _to_lhsT(wkW_m.astype(np.float32))
    wkHl = _to_lhsT(wkHp.astype(np.float32))
    f2l = _to_lhsT(F2.astype(np.float32))
    fgl = _to_lhsT(fusH_w.astype(np.float32))

    # tanh-gate folding: ACT computes tanh(0.5*v + 0.5*bv); ctx is scaled by
    # 0.5 on device, so the ctx bias constant also carries the 0.5.
    bvW = qkvW_b[1 + C:]
    bkW = qkvW_b[1:1 + C]
    bvHp = wvH @ fusW_b + qkvH_b[1 + C:]
    bkHp = wkH_m @ fusW_b + qkvH_b[1:1 + C]   # sum_h softmax = 1 -> adds to ctx
    byH = fusH_w @ fusW_b + fusH_b
    biases = np.stack([
        0.5 * bvW.reshape(2, 128), 0.5 * bkW.reshape(2, 128),
        0.5 * bvHp.reshape(2, 128), 0.5 * bkHp.reshape(2, 128),
        byH.reshape(2, 128),
    ]).astype(np.float32)

    flags = (
        not bvW.any(), not bkW.any(), not bvHp.any(), not bkHp.any(), not byH.any(),
    )
    if flags not in _BUILD_CACHE:
        _BUILD_CACHE[flags] = _build(flags)
    nc = _BUILD_CACHE[flags]

    tobf = lambda a: np.ascontiguousarray(a.astype(NPBF))
    xbf = np.ascontiguousarray(x.reshape(B, C, HW).astype(NPBF))
    in_maps = []
    for core in range(NCORES):
        in_maps.append({
            "x": xbf[core * BPC: (core + 1) * BPC],
            "statW": tobf(statW), "statH": tobf(statH),
            "wkW": tobf(wkWl), "wkH": tobf(wkHl),
            "f2": tobf(f2l), "fg": tobf(fgl),
            "biases": biases,
        })

    res = run_bass_kernel_spmd(nc, in_maps, list(range(NCORES)))
    LAST_RESULTS = res
    y = np.concatenate([r["y"] for r in res.results], axis=0)
    return y.astype(np.float32).reshape(B, C, H, W)


# revision 18
# speedup vs baseline: 1.3458x; 1.1460x over previous
"""Trainium2 Bass kernel for nn_DualAxisAggAttn (dual-axis aggregation attention).

Reference semantics per batch image x[C=256, H=64, W=64], twice (W axis then H axis):
  qkv = conv1x1(x) -> {q:[1], k:[C], v:[C]};  s = softmax_axis(q)
  ctx[c,a] = sum_r k*s;  out = x + sigmoid(v) * ctx_bcast;  y = conv1x1(out)

Distribution: data-parallel over batch (16 images -> 2 per NeuronCore x 8 cores).

Key optimizations:
  - key-path linearity: ctx = Wk @ (sum_r x*E) / S -- the key 1x1 conv moves
    AFTER the softmax-weighted reduction (N=4096 -> N=64 moving columns).
  - combine folded into the fusion matmul: ps = Wf@x + Wf@g2 (psum accum),
    so `out = x + g2` is never materialized.
  - query row replicated 128x in its m-tile -> exp(q) lands partition-broadcast.
  - sigmoid via tanh ((1+tanh(v/2))/2): exp+tanh+copy share ONE ACT table set;
    the +1 is applied in-place on the gate (4x tensor_scalar), the 0.5 folds
    into the softmax normalizer and k-bias.
  - all matmuls bf16 (host pre-casts inputs; measured rel err ~3e-3 vs fp32).
  - reductions inner-contiguous (stage W halve+reduce, stage H contiguous
    binary tree over h); no strided elementwise ops.
  - per-engine instruction streams are FIFO, so batch-stage PHASES are
    interleaved at emission time to keep the PE fed during reduce chains.
  - GPSIMD does nothing (it contends with DVE for the shared SBUF port).
"""

import numpy as np
import ml_dtypes
from contextlib import ExitStack

import concourse.bass as bass
import concourse.bacc as bacc
import concourse.tile as tile
import concourse.mybir as mybir
from concourse.bass_utils import run_bass_kernel_spmd

F32 = mybir.dt.float32
BF16 = mybir.dt.bfloat16
AF = mybir.ActivationFunctionType
ALU = mybir.AluOpType
AX = mybir.AxisListType
NPBF = ml_dtypes.bfloat16

B, C, H, W = 16, 256, 64, 64
HW = H * W
NCORES = 8
BPC = B // NCORES
KT = 2
CH = 512
NCH = HW // CH
GRP = CH // 64

_BUILD_CACHE = {}
LAST_RESULTS = None


class _Stage:
    """Emits one attention stage (one batch) in three phases."""

    def __init__(self, nc, pools, axis_w, srcs, stat, wk, fus, bias, dst_evict):
        self.nc, self.axis_w = nc, axis_w
        self.phv = pools[-1]
        self.srcs, self.stat, self.wk, self.fus = srcs, stat, wk, fus
        self.bias, self.dst_evict = bias, dst_evict
        (self.pbig, self.pgate, self.pchunk, self.pctx, self.pq, self.pv, self.pf) = pools[:7]

    def p1_alloc(self):
        self.E = self.pbig.tile([128, HW], BF16, tag="E")
        self.gate = self.pgate.tile([128, 2, HW], BF16, tag="gate")
        self.u = self.pbig.tile([128, 2, HW], BF16, tag="u")

    def p1_chunk(self, j):
        nc, srcs, stat, bias = self.nc, self.srcs, self.stat, self.bias
        E, gate, u = self.E, self.gate, self.u
        bv2 = bias.get("bv2")
        if True:
            sl = bass.ts(j, CH)
            ps_q = self.pq.tile([128, CH], F32, tag="q")
            ps_v = self.pv.tile([128, 2 * CH], F32, tag="vf")
            for kt in range(KT):
                st, sp = kt == 0, kt == KT - 1
                rhs = srcs[j][:, kt, :]
                nc.tensor.matmul(ps_q[:], stat[:, kt, 2, :], rhs, start=st, stop=sp)
                nc.tensor.matmul(ps_v[:, 0:CH], stat[:, kt, 0, :], rhs, start=st, stop=sp)
                nc.tensor.matmul(ps_v[:, CH:], stat[:, kt, 1, :], rhs, start=st, stop=sp)
            nc.scalar.activation(E[:, sl], ps_q[:], AF.Exp, bias=bias["zb"])
            if bv2 is None:
                nc.scalar.activation(
                    gate[:, :, sl], ps_v[:].rearrange("p (c n) -> p c n", c=2),
                    AF.Tanh, bias=bias["zb"], scale=0.5,
                )
            else:
                nc.scalar.activation(gate[:, 0, sl], ps_v[:, 0:CH], AF.Tanh, bias=bv2[0], scale=0.5)
                nc.scalar.activation(gate[:, 1, sl], ps_v[:, CH:], AF.Tanh, bias=bv2[1], scale=0.5)
            eb = E[:, sl].unsqueeze(1).broadcast_to([128, 2, CH])
            nc.vector.tensor_tensor(u[:, :, sl], srcs[j][:, :, :], eb, op=ALU.mult)
            if not self.axis_w:
                nc.vector.tensor_scalar_add(gate[:, :, sl], gate[:, :, sl], 1.0)

    def _reduce64(self, flat, tag):
        nc, pctx = self.nc, self.pctx
        if self.axis_w:
            v3 = flat.rearrange("p (a r) -> p a r", r=64)
            hv = self.phv.tile([128, 64, 32], BF16, tag="hv")
            nc.vector.tensor_tensor(hv[:], v3[:, :, 0:32], v3[:, :, 32:64], op=ALU.add)
            h2 = self.phv.tile([128, 64, 16], BF16, tag="hv2")
            nc.vector.tensor_tensor(h2[:], hv[:, :, 0:16], hv[:, :, 16:32], op=ALU.add)
            out = pctx.tile([128, 64], F32, tag=f"red_{tag}")
            nc.vector.tensor_reduce(out[:], h2[:], axis=AX.X, op=ALU.add)
        else:
            t = self.phv.tile([128, 2048], BF16, tag="tree2")
            nc.vector.tensor_tensor(t[:], flat[:, 0:2048], flat[:, 2048:4096], op=ALU.add)
            n = 1024
            while n >= 128:
                nc.vector.tensor_tensor(t[:, 0:n], t[:, 0:n], t[:, n : 2 * n], op=ALU.add)
                n //= 2
            out = pctx.tile([128, 64], F32, tag=f"red_{tag}")
            nc.vector.tensor_tensor(out[:], t[:, 0:64], t[:, 64:128], op=ALU.add)
        return out

    def p2(self):
        nc, pctx, bias = self.nc, self.pctx, self.bias
        S = self._reduce64(self.E[:], "S")
        R = pctx.tile([128, 64], F32, tag="R")
        nc.vector.reciprocal(R[:], S[:])
        xen = []
        if self.axis_w:
            xes = [self._reduce64(self.u[:, ct, :], f"xe{ct}") for ct in range(2)]
        else:
            # merged tree over both c-tiles: [128, 2, n] contiguous views
            t = self.phv.tile([128, 2, 2048], BF16, tag="tree2")
            u = self.u
            nc.vector.tensor_tensor(t[:], u[:, :, 0:2048], u[:, :, 2048:4096], op=ALU.add)
            n = 1024
            while n >= 128:
                nc.vector.tensor_tensor(t[:, :, 0:n], t[:, :, 0:n], t[:, :, n:2*n], op=ALU.add)
                n //= 2
            xep = self.pctx.tile([128, 2, 64], F32, tag="xep")
            nc.vector.tensor_tensor(xep[:], t[:, :, 0:64], t[:, :, 64:128], op=ALU.add)
            xes = [xep[:, ct, :] for ct in range(2)]
        for ct in range(2):
            xn = pctx.tile([128, 64], BF16, tag=f"xn{ct}")
            nc.vector.tensor_tensor(xn[:], xes[ct], R[:], op=ALU.mult)
            xen.append(xn)
        self.ctxs = []
        bk2 = bias.get("bk2")
        for mt in range(2):
            ps_c = self.pq.tile([128, 64], F32, tag="q")
            for ct in range(2):
                nc.tensor.matmul(ps_c[:], self.wk[:, ct, mt, :], xen[ct][:], start=ct == 0, stop=ct == 1)
            cn = pctx.tile([128, 64], BF16, tag=f"cn{mt}")
            if bk2 is None:
                nc.vector.tensor_scalar_mul(cn[:], ps_c[:], 0.5)
            else:
                nc.vector.tensor_scalar(cn[:], ps_c[:], 0.5, bk2[mt], op0=ALU.mult, op1=ALU.add)
            self.ctxs.append(cn)

    def p3_chunk(self, j):
        nc, srcs, fus = self.nc, self.srcs, self.fus
        if True:
            sl = bass.ts(j, CH)
            g2s = []
            for ct in range(2):
                if self.axis_w:
                    cb = self.ctxs[ct][:, bass.ts(j, GRP)].unsqueeze(2).broadcast_to([128, GRP, 64])
                else:
                    cb = self.ctxs[ct][:].unsqueeze(1).broadcast_to([128, GRP, 64])
                g2 = self.pchunk.tile([128, GRP, 64], BF16, tag=f"g2_{ct}")
                gv = self.gate[:, ct, sl].rearrange("p (a r) -> p a r", r=64)
                if self.axis_w:
                    nc.vector.scalar_tensor_tensor(g2[:], gv, 1.0, cb, op0=ALU.add, op1=ALU.mult)
                else:
                    nc.vector.tensor_tensor(g2[:], gv, cb, op=ALU.mult)
                g2s.append(g2)
            ps_f = self.pf.tile([128, 2 * CH], F32, tag="vf")
            for mt in range(2):
                half = ps_f[:, bass.ts(mt, CH)]
                nc.tensor.matmul(half, fus[:, 0, mt, :], srcs[j][:, 0, :], start=True, stop=False)
                nc.tensor.matmul(half, fus[:, 1, mt, :], srcs[j][:, 1, :], start=False, stop=False)
                nc.tensor.matmul(half, fus[:, 0, mt, :], g2s[0][:].rearrange("p a r -> p (a r)"), start=False, stop=False)
                nc.tensor.matmul(half, fus[:, 1, mt, :], g2s[1][:].rearrange("p a r -> p (a r)"), start=False, stop=True)
            self.dst_evict(j, ps_f)


def _build(flags):
    bvW0, bkW0, bvH0, bkH0, bfW0, bfH0 = flags
    nc = bacc.Bacc(trn_type="TRN2", target_bir_lowering=False, debug=False)

    x_d = nc.dram_tensor("x", [BPC, C, HW], BF16, kind="ExternalInput").ap()
    statW_d = nc.dram_tensor("statW", [128, KT, 3, 128], BF16, kind="ExternalInput").ap()
    statH_d = nc.dram_tensor("statH", [128, KT, 3, 128], BF16, kind="ExternalInput").ap()
    wkW_d = nc.dram_tensor("wkW", [128, KT, 2, 128], BF16, kind="ExternalInput").ap()
    wkH_d = nc.dram_tensor("wkH", [128, KT, 2, 128], BF16, kind="ExternalInput").ap()
    fusW_d = nc.dram_tensor("fusW", [128, KT, 2, 128], BF16, kind="ExternalInput").ap()
    fusH_d = nc.dram_tensor("fusH", [128, KT, 2, 128], BF16, kind="ExternalInput").ap()
    bias_d = nc.dram_tensor("biases", [6, 2, 128], F32, kind="ExternalInput").ap()
    y_d = nc.dram_tensor("y", [BPC, C, HW], F32, kind="ExternalOutput").ap()

    with tile.TileContext(nc) as tc, ExitStack() as ctx:
        wp = ctx.enter_context(tc.tile_pool(name="weights", bufs=1))
        xbp = ctx.enter_context(tc.tile_pool(name="xbf", bufs=18))
        xwp = ctx.enter_context(tc.tile_pool(name="xw", bufs=16))
        pbig = ctx.enter_context(tc.tile_pool(name="big", bufs=2))
        pgate = ctx.enter_context(tc.tile_pool(name="gate", bufs=2))
        pchunk = ctx.enter_context(tc.tile_pool(name="chunk", bufs=3))
        pctx = ctx.enter_context(tc.tile_pool(name="ctx", bufs=3))
        phv = ctx.enter_context(tc.tile_pool(name="hv", bufs=2))
        yp = ctx.enter_context(tc.tile_pool(name="yev", bufs=3))
        pq = ctx.enter_context(tc.tile_pool(name="psq", bufs=2, space="PSUM"))
        pvf = ctx.enter_context(tc.tile_pool(name="psvf", bufs=3, space="PSUM"))
        pools = (pbig, pgate, pchunk, pctx, pq, pvf, pvf, phv)

        def wload(name, dram, shape, dt):
            t = wp.tile(shape, dt, tag=name)
            nc.scalar.dma_start(t[:], dram[:])
            return t

        statW = wload("statW", statW_d, [128, KT, 3, 128], BF16)
        statH = wload("statH", statH_d, [128, KT, 3, 128], BF16)
        wkW = wload("wkW", wkW_d, [128, KT, 2, 128], BF16)
        wkH = wload("wkH", wkH_d, [128, KT, 2, 128], BF16)
        fusW = wload("fusW", fusW_d, [128, KT, 2, 128], BF16)
        fusH = wload("fusH", fusH_d, [128, KT, 2, 128], BF16)

        bias_sb = wp.tile([128, 6, 2], F32, tag="biases")
        nc.scalar.dma_start(bias_sb[:], bias_d[:].transpose([2, 0, 1]))
        zb = wp.tile([128, 1], F32, tag="zb")
        nc.vector.memset(zb[:], 0.0)

        def bap(i, ct):
            return bias_sb[:, i, ct].unsqueeze(1)

        biasW = {
            "bv2": None if bvW0 else [bap(0, ct) for ct in range(2)],
            "bk2": None if bkW0 else [bap(1, ct) for ct in range(2)],
            "zb": zb[:],
        }
        biasH = {
            "bv2": None if bvH0 else [bap(2, ct) for ct in range(2)],
            "bk2": None if bkH0 else [bap(3, ct) for ct in range(2)],
            "zb": zb[:],
        }

        def load_x(b):
            xcs = []
            for j in range(NCH):
                xc = xbp.tile([128, KT, CH], BF16, tag="xc")
                for kt in range(KT):
                    nc.sync.dma_start(xc[:, kt, :], x_d[b, bass.ts(kt, 128), bass.ts(j, CH)])
                xcs.append(xc[:])
            return xcs

        def make_W(b, xcs):
            xw_tiles = [None] * NCH

            def evW(j, ps_f):
                xw = xwp.tile([128, KT, CH], BF16, tag="xw")
                xw_tiles[j] = xw[:]
                if bfW0:
                    nc.scalar.activation(xw[:], ps_f[:].rearrange("p (c n) -> p c n", c=2), AF.Copy)
                else:
                    for ct in range(2):
                        nc.scalar.activation(
                            xw[:, ct, :], ps_f[:, bass.ts(ct, CH)],
                            AF.Identity, bias=bap(4, ct),
                        )

            st = _Stage(nc, pools, True, xcs, statW, wkW, fusW, biasW, evW)
            st.xw_tiles = xw_tiles
            return st

        def make_H(b, xw_tiles):
            def evH(j, ps_f):
                y_t = yp.tile([128, 2, CH], F32, tag="y")
                if bfH0:
                    nc.scalar.activation(y_t[:], ps_f[:].rearrange("p (c n) -> p c n", c=2), AF.Copy)
                else:
                    for ct in range(2):
                        nc.scalar.activation(
                            y_t[:, ct, :], ps_f[:, bass.ts(ct, CH)],
                            AF.Identity, bias=bap(5, ct),
                        )
                nc.sync.dma_start(
                    y_d[b].rearrange("(c p) n -> p c n", p=128)[:, :, bass.ts(j, CH)],
                    y_t[:],
                )

            return _Stage(nc, pools, False, xw_tiles, statH, wkH, fusH, biasH, evH)

        # interleaved phase schedule: chunk-level alternation keeps every
        # engine's FIFO stream fed during the other phase's stalls
        def run_p1(st):
            st.p1_alloc()
            for j in range(NCH):
                st.p1_chunk(j)

        def run_p3(st):
            for j in range(NCH):
                st.p3_chunk(j)

        x0 = load_x(0)
        x1 = load_x(1)
        w0 = make_W(0, x0)
        w1 = make_W(1, x1)
        run_p1(w0)
        run_p1(w1)
        w0.p2()
        run_p3(w0)
        h0 = make_H(0, w0.xw_tiles)
        w1.p2()
        run_p1(h0)
        run_p3(w1)
        h1 = make_H(1, w1.xw_tiles)
        h0.p2()
        run_p1(h1)
        run_p3(h0)
        h1.p2()
        run_p3(h1)

    nc.compile()
    return nc


def _prep(qkv_w, fus_w):
    wq = qkv_w[0]
    wk = qkv_w[1 : 1 + C]
    wv = qkv_w[1 + C :]
    stat = np.empty((128, KT, 3, 128), np.float32)
    wkt = np.empty((128, KT, 2, 128), np.float32)
    fus = np.empty((128, KT, 2, 128), np.float32)
    for kt in range(KT):
        cs = slice(kt * 128, (kt + 1) * 128)
        stat[:, kt, 0, :] = wv[0:128, cs].T
        stat[:, kt, 1, :] = wv[128:256, cs].T
        stat[:, kt, 2, :] = np.repeat(wq[cs][:, None], 128, axis=1)
        wkt[:, kt, 0, :] = wk[0:128, cs].T
        wkt[:, kt, 1, :] = wk[128:256, cs].T
        fus[:, kt, 0, :] = fus_w[0:128, cs].T
        fus[:, kt, 1, :] = fus_w[128:256, cs].T
    tobf = lambda a: np.ascontiguousarray(a.astype(NPBF))
    return tobf(stat), tobf(wkt), tobf(fus)


def kernel(x, qkvW_w, qkvW_b, qkvH_w, qkvH_b, fusW_w, fusW_b, fusH_w, fusH_b):
    global LAST_RESULTS
    x = np.asarray(x, np.float32)
    qkvW_w = np.asarray(qkvW_w, np.float32)
    qkvW_b = np.asarray(qkvW_b, np.float32)
    qkvH_w = np.asarray(qkvH_w, np.float32)
    qkvH_b = np.asarray(qkvH_b, np.float32)
    fusW_w = np.asarray(fusW_w, np.float32)
    fusW_b = np.asarray(fusW_b, np.float32)
    fusH_w = np.asarray(fusH_w, np.float32)
    fusH_b = np.asarray(fusH_b, np.float32)

    statW, wkW, fusW = _prep(qkvW_w, fusW_w)
    statH, wkH, fusH = _prep(qkvH_w, fusH_w)

    bkW = qkvW_b[1 : 1 + C]
    bvW = qkvW_b[1 + C :]
    bkH = qkvH_b[1 : 1 + C]
    bvH = qkvH_b[1 + C :]
    biases = np.stack(
        [0.5 * bvW.reshape(2, 128),
         0.5 * bkW.reshape(2, 128),
         0.5 * bvH.reshape(2, 128),
         0.5 * bkH.reshape(2, 128),
         fusW_b.reshape(2, 128),
         fusH_b.reshape(2, 128)]
    ).astype(np.float32)

    flags = (
        not bvW.any(), not bkW.any(), not bvH.any(), not bkH.any(),
        not fusW_b.any(), not fusH_b.any(),
    )
    if flags not in _BUILD_CACHE:
        _BUILD_CACHE[flags] = _build(flags)
    nc = _BUILD_CACHE[flags]

    xbf = np.ascontiguousarray(x.reshape(B, C, HW).astype(NPBF))
    in_maps = []
    for core in range(NCORES):
        in_maps.append({
            "x": xbf[core * BPC : (core + 1) * BPC],
            "statW": statW, "statH": statH,
            "wkW": wkW, "wkH": wkH, "fusW": fusW, "fusH": fusH,
            "biases": biases,
        })

    res = run_bass_kernel_spmd(nc, in_maps, list(range(NCORES)))
    LAST_RESULTS = res
    y = np.concatenate([r["y"] for r in res.results], axis=0)
    return y.reshape(B, C, H, W)



# revision 19
# speedup vs baseline: 1.3520x; 1.0046x over previous
"""Trainium2 Bass kernel for nn_DualAxisAggAttn (dual-axis aggregation attention).

Reference semantics per batch image x[C=256, H=64, W=64], twice (W axis then H axis):
  qkv = conv1x1(x) -> {q:[1], k:[C], v:[C]};  s = softmax_axis(q)
  ctx[c,a] = sum_r k*s;  out = x + sigmoid(v) * ctx_bcast;  y = conv1x1(out)

Distribution: data-parallel over batch (16 images -> 2 per NeuronCore x 8 cores).

Key optimizations:
  - key-path linearity: ctx = Wk @ (sum_r x*E) / S -- the key 1x1 conv moves
    AFTER the softmax-weighted reduction (N=4096 -> N=64 moving columns).
  - combine folded into the fusion matmul: ps = Wf@x + Wf@g2 (psum accum),
    so `out = x + g2` is never materialized.
  - query row replicated 128x in its m-tile -> exp(q) lands partition-broadcast.
  - sigmoid via tanh ((1+tanh(v/2))/2): exp+tanh+copy share ONE ACT table set;
    the +1 is applied in-place on the gate (4x tensor_scalar), the 0.5 folds
    into the softmax normalizer and k-bias.
  - all matmuls bf16 (host pre-casts inputs; measured rel err ~3e-3 vs fp32).
  - reductions inner-contiguous (stage W halve+reduce, stage H contiguous
    binary tree over h); no strided elementwise ops.
  - per-engine instruction streams are FIFO, so batch-stage PHASES are
    interleaved at emission time to keep the PE fed during reduce chains.
  - GPSIMD does nothing (it contends with DVE for the shared SBUF port).
"""

import numpy as np
import ml_dtypes
from contextlib import ExitStack

import concourse.bass as bass
import concourse.bacc as bacc
import concourse.tile as tile
import concourse.mybir as mybir
from concourse.bass_utils import run_bass_kernel_spmd

F32 = mybir.dt.float32
BF16 = mybir.dt.bfloat16
AF = mybir.ActivationFunctionType
ALU = mybir.AluOpType
AX = mybir.AxisListType
NPBF = ml_dtypes.bfloat16

B, C, H, W = 16, 256, 64, 64
HW = H * W
NCORES = 8
BPC = B // NCORES
KT = 2
CH = 512
NCH = HW // CH
GRP = CH // 64

_BUILD_CACHE = {}
LAST_RESULTS = None


class _Stage:
    """Emits one attention stage (one batch) in three phases."""

    def __init__(self, nc, pools, axis_w, srcs, stat, wk, fus, bias, dst_evict):
        self.nc, self.axis_w = nc, axis_w
        self.phv = pools[-1]
        self.srcs, self.stat, self.wk, self.fus = srcs, stat, wk, fus
        self.bias, self.dst_evict = bias, dst_evict
        (self.pbig, self.pgate, self.pchunk, self.pctx, self.pq, self.pv, self.pf) = pools[:7]

    def p1_alloc(self):
        self.E = self.pbig.tile([128, HW], BF16, tag="E")
        self.gate = self.pgate.tile([128, 2, HW], BF16, tag="gate")
        self.u = self.pbig.tile([128, 2, HW], BF16, tag="u")

    def p1_chunk(self, j):
        nc, srcs, stat, bias = self.nc, self.srcs, self.stat, self.bias
        E, gate, u = self.E, self.gate, self.u
        bv2 = bias.get("bv2")
        if True:
            sl = bass.ts(j, CH)
            ps_q = self.pq.tile([128, CH], F32, tag="q")
            ps_v = self.pv.tile([128, 2 * CH], F32, tag="vf")
            for kt in range(KT):
                st, sp = kt == 0, kt == KT - 1
                rhs = srcs[j][:, kt, :]
                nc.tensor.matmul(ps_q[:], stat[:, kt, 2, :], rhs, start=st, stop=sp)
                nc.tensor.matmul(ps_v[:, 0:CH], stat[:, kt, 0, :], rhs, start=st, stop=sp)
                nc.tensor.matmul(ps_v[:, CH:], stat[:, kt, 1, :], rhs, start=st, stop=sp)
            nc.scalar.activation(E[:, sl], ps_q[:], AF.Exp, bias=bias["zb"])
            if bv2 is None:
                nc.scalar.activation(
                    gate[:, :, sl], ps_v[:].rearrange("p (c n) -> p c n", c=2),
                    AF.Tanh, bias=bias["zb"], scale=0.5,
                )
            else:
                nc.scalar.activation(gate[:, 0, sl], ps_v[:, 0:CH], AF.Tanh, bias=bv2[0], scale=0.5)
                nc.scalar.activation(gate[:, 1, sl], ps_v[:, CH:], AF.Tanh, bias=bv2[1], scale=0.5)
            eb = E[:, sl].unsqueeze(1).broadcast_to([128, 2, CH])
            nc.vector.tensor_tensor(u[:, :, sl], srcs[j][:, :, :], eb, op=ALU.mult)
            if not self.axis_w:
                nc.vector.tensor_scalar_add(gate[:, :, sl], gate[:, :, sl], 1.0)

    def _reduce64(self, flat, tag):
        nc, pctx = self.nc, self.pctx
        if self.axis_w:
            v3 = flat.rearrange("p (a r) -> p a r", r=64)
            hv = self.phv.tile([128, 64, 32], BF16, tag="hv")
            nc.vector.tensor_tensor(hv[:], v3[:, :, 0:32], v3[:, :, 32:64], op=ALU.add)
            h2 = self.phv.tile([128, 64, 16], BF16, tag="hv2")
            nc.vector.tensor_tensor(h2[:], hv[:, :, 0:16], hv[:, :, 16:32], op=ALU.add)
            nc.vector.tensor_tensor(h2[:, :, 0:8], h2[:, :, 0:8], h2[:, :, 8:16], op=ALU.add)
            out = pctx.tile([128, 64], F32, tag=f"red_{tag}")
            nc.vector.tensor_reduce(out[:], h2[:, :, 0:8], axis=AX.X, op=ALU.add)
        else:
            t = self.phv.tile([128, 2048], BF16, tag="tree2")
            nc.vector.tensor_tensor(t[:], flat[:, 0:2048], flat[:, 2048:4096], op=ALU.add)
            n = 1024
            while n >= 128:
                nc.vector.tensor_tensor(t[:, 0:n], t[:, 0:n], t[:, n : 2 * n], op=ALU.add)
                n //= 2
            out = pctx.tile([128, 64], F32, tag=f"red_{tag}")
            nc.vector.tensor_tensor(out[:], t[:, 0:64], t[:, 64:128], op=ALU.add)
        return out

    def p2(self):
        nc, pctx, bias = self.nc, self.pctx, self.bias
        S = self._reduce64(self.E[:], "S")
        R = pctx.tile([128, 64], F32, tag="R")
        nc.vector.reciprocal(R[:], S[:])
        xen = []
        if self.axis_w:
            xes = [self._reduce64(self.u[:, ct, :], f"xe{ct}") for ct in range(2)]
        else:
            # merged tree over both c-tiles: [128, 2, n] contiguous views
            t = self.phv.tile([128, 2, 2048], BF16, tag="tree2")
            u = self.u
            nc.vector.tensor_tensor(t[:], u[:, :, 0:2048], u[:, :, 2048:4096], op=ALU.add)
            n = 1024
            while n >= 128:
                nc.vector.tensor_tensor(t[:, :, 0:n], t[:, :, 0:n], t[:, :, n:2*n], op=ALU.add)
                n //= 2
            xep = self.pctx.tile([128, 2, 64], F32, tag="xep")
            nc.vector.tensor_tensor(xep[:], t[:, :, 0:64], t[:, :, 64:128], op=ALU.add)
            xes = [xep[:, ct, :] for ct in range(2)]
        for ct in range(2):
            xn = pctx.tile([128, 64], BF16, tag=f"xn{ct}")
            nc.vector.tensor_tensor(xn[:], xes[ct], R[:], op=ALU.mult)
            xen.append(xn)
        self.ctxs = []
        bk2 = bias.get("bk2")
        for mt in range(2):
            ps_c = self.pq.tile([128, 64], F32, tag="q")
            for ct in range(2):
                nc.tensor.matmul(ps_c[:], self.wk[:, ct, mt, :], xen[ct][:], start=ct == 0, stop=ct == 1)
            cn = pctx.tile([128, 64], BF16, tag=f"cn{mt}")
            if bk2 is None:
                nc.vector.tensor_scalar_mul(cn[:], ps_c[:], 0.5)
            else:
                nc.vector.tensor_scalar(cn[:], ps_c[:], 0.5, bk2[mt], op0=ALU.mult, op1=ALU.add)
            self.ctxs.append(cn)

    def p3_chunk(self, j):
        nc, srcs, fus = self.nc, self.srcs, self.fus
        if True:
            sl = bass.ts(j, CH)
            g2s = []
            for ct in range(2):
                if self.axis_w:
                    cb = self.ctxs[ct][:, bass.ts(j, GRP)].unsqueeze(2).broadcast_to([128, GRP, 64])
                else:
                    cb = self.ctxs[ct][:].unsqueeze(1).broadcast_to([128, GRP, 64])
                g2 = self.pchunk.tile([128, GRP, 64], BF16, tag=f"g2_{ct}")
                gv = self.gate[:, ct, sl].rearrange("p (a r) -> p a r", r=64)
                if self.axis_w:
                    nc.vector.scalar_tensor_tensor(g2[:], gv, 1.0, cb, op0=ALU.add, op1=ALU.mult)
                else:
                    nc.vector.tensor_tensor(g2[:], gv, cb, op=ALU.mult)
                g2s.append(g2)
            ps_f = self.pf.tile([128, 2 * CH], F32, tag="vf")
            for mt in range(2):
                half = ps_f[:, bass.ts(mt, CH)]
                nc.tensor.matmul(half, fus[:, 0, mt, :], srcs[j][:, 0, :], start=True, stop=False)
                nc.tensor.matmul(half, fus[:, 1, mt, :], srcs[j][:, 1, :], start=False, stop=False)
                nc.tensor.matmul(half, fus[:, 0, mt, :], g2s[0][:].rearrange("p a r -> p (a r)"), start=False, stop=False)
                nc.tensor.matmul(half, fus[:, 1, mt, :], g2s[1][:].rearrange("p a r -> p (a r)"), start=False, stop=True)
            self.dst_evict(j, ps_f)


def _build(flags):
    bvW0, bkW0, bvH0, bkH0, bfW0, bfH0 = flags
    nc = bacc.Bacc(trn_type="TRN2", target_bir_lowering=False, debug=False)

    x_d = nc.dram_tensor("x", [BPC, C, HW], BF16, kind="ExternalInput").ap()
    statW_d = nc.dram_tensor("statW", [128, KT, 3, 128], BF16, kind="ExternalInput").ap()
    statH_d = nc.dram_tensor("statH", [128, KT, 3, 128], BF16, kind="ExternalInput").ap()
    wkW_d = nc.dram_tensor("wkW", [128, KT, 2, 128], BF16, kind="ExternalInput").ap()
    wkH_d = nc.dram_tensor("wkH", [128, KT, 2, 128], BF16, kind="ExternalInput").ap()
    fusW_d = nc.dram_tensor("fusW", [128, KT, 2, 128], BF16, kind="ExternalInput").ap()
    fusH_d = nc.dram_tensor("fusH", [128, KT, 2, 128], BF16, kind="ExternalInput").ap()
    bias_d = nc.dram_tensor("biases", [6, 2, 128], F32, kind="ExternalInput").ap()
    y_d = nc.dram_tensor("y", [BPC, C, HW], BF16, kind="ExternalOutput").ap()

    with tile.TileContext(nc) as tc, ExitStack() as ctx:
        wp = ctx.enter_context(tc.tile_pool(name="weights", bufs=1))
        xbp = ctx.enter_context(tc.tile_pool(name="xbf", bufs=18))
        xwp = ctx.enter_context(tc.tile_pool(name="xw", bufs=16))
        pbig = ctx.enter_context(tc.tile_pool(name="big", bufs=2))
        pgate = ctx.enter_context(tc.tile_pool(name="gate", bufs=2))
        pchunk = ctx.enter_context(tc.tile_pool(name="chunk", bufs=3))
        pctx = ctx.enter_context(tc.tile_pool(name="ctx", bufs=3))
        phv = ctx.enter_context(tc.tile_pool(name="hv", bufs=2))
        yp = ctx.enter_context(tc.tile_pool(name="yev", bufs=3))
        pq = ctx.enter_context(tc.tile_pool(name="psq", bufs=2, space="PSUM"))
        pvf = ctx.enter_context(tc.tile_pool(name="psvf", bufs=3, space="PSUM"))
        pools = (pbig, pgate, pchunk, pctx, pq, pvf, pvf, phv)

        def wload(name, dram, shape, dt):
            t = wp.tile(shape, dt, tag=name)
            nc.scalar.dma_start(t[:], dram[:])
            return t

        statW = wload("statW", statW_d, [128, KT, 3, 128], BF16)
        statH = wload("statH", statH_d, [128, KT, 3, 128], BF16)
        wkW = wload("wkW", wkW_d, [128, KT, 2, 128], BF16)
        wkH = wload("wkH", wkH_d, [128, KT, 2, 128], BF16)
        fusW = wload("fusW", fusW_d, [128, KT, 2, 128], BF16)
        fusH = wload("fusH", fusH_d, [128, KT, 2, 128], BF16)

        bias_sb = wp.tile([128, 6, 2], F32, tag="biases")
        nc.scalar.dma_start(bias_sb[:], bias_d[:].transpose([2, 0, 1]))
        zb = wp.tile([128, 1], F32, tag="zb")
        nc.vector.memset(zb[:], 0.0)

        def bap(i, ct):
            return bias_sb[:, i, ct].unsqueeze(1)

        biasW = {
            "bv2": None if bvW0 else [bap(0, ct) for ct in range(2)],
            "bk2": None if bkW0 else [bap(1, ct) for ct in range(2)],
            "zb": zb[:],
        }
        biasH = {
            "bv2": None if bvH0 else [bap(2, ct) for ct in range(2)],
            "bk2": None if bkH0 else [bap(3, ct) for ct in range(2)],
            "zb": zb[:],
        }

        def load_x(b):
            xcs = []
            for j in range(NCH):
                xc = xbp.tile([128, KT, CH], BF16, tag="xc")
                for kt in range(KT):
                    nc.sync.dma_start(xc[:, kt, :], x_d[b, bass.ts(kt, 128), bass.ts(j, CH)])
                xcs.append(xc[:])
            return xcs

        def make_W(b, xcs):
            xw_tiles = [None] * NCH

            def evW(j, ps_f):
                xw = xwp.tile([128, KT, CH], BF16, tag="xw")
                xw_tiles[j] = xw[:]
                if bfW0:
                    nc.scalar.activation(xw[:], ps_f[:].rearrange("p (c n) -> p c n", c=2), AF.Copy)
                else:
                    for ct in range(2):
                        nc.scalar.activation(
                            xw[:, ct, :], ps_f[:, bass.ts(ct, CH)],
                            AF.Identity, bias=bap(4, ct),
                        )

            st = _Stage(nc, pools, True, xcs, statW, wkW, fusW, biasW, evW)
            st.xw_tiles = xw_tiles
            return st

        def make_H(b, xw_tiles):
            def evH(j, ps_f):
                y_t = yp.tile([128, 2, CH], BF16, tag="y")
                if bfH0:
                    nc.scalar.activation(y_t[:], ps_f[:].rearrange("p (c n) -> p c n", c=2), AF.Copy)
                else:
                    for ct in range(2):
                        nc.scalar.activation(
                            y_t[:, ct, :], ps_f[:, bass.ts(ct, CH)],
                            AF.Identity, bias=bap(5, ct),
                        )
                nc.sync.dma_start(
                    y_d[b].rearrange("(c p) n -> p c n", p=128)[:, :, bass.ts(j, CH)],
                    y_t[:],
                )

            return _Stage(nc, pools, False, xw_tiles, statH, wkH, fusH, biasH, evH)

        # interleaved phase schedule: chunk-level alternation keeps every
        # engine's FIFO stream fed during the other phase's stalls
        def run_p1(st):
            st.p1_alloc()
            for j in range(NCH):
                st.p1_chunk(j)

        def run_p3(st):
            for j in range(NCH):
                st.p3_chunk(j)

        x0 = load_x(0)
        x1 = load_x(1)
        w0 = make_W(0, x0)
        w1 = make_W(1, x1)
        run_p1(w0)
        run_p1(w1)
        w0.p2()
        run_p3(w0)
        h0 = make_H(0, w0.xw_tiles)
        w1.p2()
        run_p1(h0)
        run_p3(w1)
        h1 = make_H(1, w1.xw_tiles)
        h0.p2()
        run_p1(h1)
        run_p3(h0)
        h1.p2()
        run_p3(h1)

    nc.compile()
    return nc


def _prep(qkv_w, fus_w):
    wq = qkv_w[0]
    wk = qkv_w[1 : 1 + C]
    wv = qkv_w[1 + C :]
    stat = np.empty((128, KT, 3, 128), np.float32)
    wkt = np.empty((128, KT, 2, 128), np.float32)
    fus = np.empty((128, KT, 2, 128), np.float32)
    for kt in range(KT):
        cs = slice(kt * 128, (kt + 1) * 128)
        stat[:, kt, 0, :] = wv[0:128, cs].T
        stat[:, kt, 1, :] = wv[128:256, cs].T
        stat[:, kt, 2, :] = np.repeat(wq[cs][:, None], 128, axis=1)
        wkt[:, kt, 0, :] = wk[0:128, cs].T
        wkt[:, kt, 1, :] = wk[128:256, cs].T
        fus[:, kt, 0, :] = fus_w[0:128, cs].T
        fus[:, kt, 1, :] = fus_w[128:256, cs].T
    tobf = lambda a: np.ascontiguousarray(a.astype(NPBF))
    return tobf(stat), tobf(wkt), tobf(fus)


def kernel(x, qkvW_w, qkvW_b, qkvH_w, qkvH_b, fusW_w, fusW_b, fusH_w, fusH_b):
    global LAST_RESULTS
    x = np.asarray(x, np.float32)
    qkvW_w = np.asarray(qkvW_w, np.float32)
    qkvW_b = np.asarray(qkvW_b, np.float32)
    qkvH_w = np.asarray(qkvH_w, np.float32)
    qkvH_b = np.asarray(qkvH_b, np.float32)
    fusW_w = np.asarray(fusW_w, np.float32)
    fusW_b = np.asarray(fusW_b, np.float32)
    fusH_w = np.asarray(fusH_w, np.float32)
    fusH_b = np.asarray(fusH_b, np.float32)

    statW, wkW, fusW = _prep(qkvW_w, fusW_w)
    statH, wkH, fusH = _prep(qkvH_w, fusH_w)

    bkW = qkvW_b[1 : 1 + C]
    bvW = qkvW_b[1 + C :]
    bkH = qkvH_b[1 : 1 + C]
    bvH = qkvH_b[1 + C :]
    biases = np.stack(
        [0.5 * bvW.reshape(2, 128),
         0.5 * bkW.reshape(2, 128),
         0.5 * bvH.reshape(2, 128),
         0.5 * bkH.reshape(2, 128),
         fusW_b.reshape(2, 128),
         fusH_b.reshape(2, 128)]
    ).astype(np.float32)

    flags = (
        not bvW.any(), not bkW.any(), not bvH.any(), not bkH.any(),
        not fusW_b.any(), not fusH_b.any(),
    )
    if flags not in _BUILD_CACHE:
        _BUILD_CACHE[flags] = _build(flags)
    nc = _BUILD_CACHE[flags]

    xbf = np.ascontiguousarray(x.reshape(B, C, HW).astype(NPBF))
    in_maps = []
    for core in range(NCORES):
        in_maps.append({
            "x": xbf[core * BPC : (core + 1) * BPC],
            "statW": statW, "statH": statH,
            "wkW": wkW, "wkH": wkH, "fusW": fusW, "fusH": fusH,
            "biases": biases,
        })

    res = run_bass_kernel_spmd(nc, in_maps, list(range(NCORES)))
    LAST_RESULTS = res
    y = np.concatenate([r["y"] for r in res.results], axis=0)
    return y.astype(np.float32).reshape(B, C, H, W)

